# revision 2
# baseline (speedup 1.0000x reference)
"""ChebyKAN layer on 8 Trainium2 NeuronCores.

y[b,o] = sum_{i,d} T_d(tanh(x[b,i])) * coeffs[i,o,d]

The Chebyshev basis is re-parameterized (exact 9x9 linear transform of the
coefficients on host) into products of Chebyshev values built with ACT
squares + DVE fused ops per element:
  G0=1, G1=t, G2=t^2, G3q=(G2-3/4)t=T3/4, G4=(2G2-1)^2=T2^2,
  G5q=(G4-1/2)t=(T5+T3)/4, G6=(4G3q)^2=T3^2, G7q=(G6-1/2)t=(T7+T5)/4,
  G8=(2G4-1)^2=T4^2

G0=1 contributes a batch-independent bias sum_i c'[i,o,0], folded in on the
host and added during PSUM eviction — the device contraction is only levels
1..8: a (B x 8192) @ (8192 x 1024) matmul in bf16 (1 cycle/row, fast
weight load), accumulated in fp32 PSUM over 64 contraction blocks.

Layout per core (2048 batch rows): 4 macros of 512 rows. Per macro all 64
G-tiles [128, 512] bf16 are resident; the k-loop is i-block-major and for
each k streams one [128, 1024] bf16 coeff block, issuing 4(bt) x 2(oh)
matmuls into 8 PSUM banks. Each stationary G block feeds 2 consecutive
matmuls. Basis for macro m+1 is emitted i-block-by-i-block as macro m's
k-loop retires the matching i-block, so ACT/DVE overlap the PE stream.

Sharding: data-parallel over batch (2048 rows/core), coeffs replicated.
"""

import numpy as np
import concourse.mybir as mybir
import concourse.tile as tile
from concourse import bacc
from concourse.bass_utils import run_bass_kernel_spmd

B, I, O = 16384, 1024, 1024
D1 = 9
CORES = 8
BC = B // CORES            # 2048 batch rows per core
P = 128
MACRO = 512                # batch rows per macro generation
NM = BC // MACRO           # 4 macros
OH = 512                   # psum bank width
IB = I // P                # 8 i-blocks
LEV = 8                    # device levels (G1..G8)
NK = LEV * IB              # 64 contraction blocks
BT = MACRO // P            # 4 batch subtiles per macro

F32 = mybir.dt.float32
BF16 = mybir.dt.bfloat16
AF = mybir.ActivationFunctionType
OP = mybir.AluOpType

_CACHE = {}
_last_in_maps = None

# G_k = sum_d M[k,d] T_d  (exact); host solves M^T C' = C
_M = np.zeros((9, 9))
_M[0, 0] = 1; _M[1, 1] = 1
_M[2, 0] = .5; _M[2, 2] = .5
_M[3, 3] = .25
_M[4, 0] = .5; _M[4, 4] = .5
_M[5, 3] = .25; _M[5, 5] = .25
_M[6, 0] = .5; _M[6, 6] = .5
_M[7, 5] = .25; _M[7, 7] = .25
_M[8, 0] = .5; _M[8, 8] = .5
_A = np.linalg.inv(_M.T)


def _basis(nc, xp, fp, gp, xt_d, neg1, gm, ib, g):
    """Emit G1..G8 [128, MACRO] bf16 tiles for (macro gm, i-block ib)."""
    m = gm % NM
    sfx = f"{gm}_{ib}"
    xt = xp.tile([P, MACRO], F32, tag="xt", name=f"xt_{sfx}")
    nc.sync.dma_start(xt[:], xt_d[ib * P:(ib + 1) * P, m * MACRO:(m + 1) * MACRO])

    def gt(lev):
        w = gp.tile([P, MACRO], BF16, tag=f"g{lev}_{ib}", name=f"g{lev}_{ib}_{gm}")
        g[(gm, lev, ib)] = w
        return w

    t_f = fp.tile([P, MACRO], F32, tag="t", name=f"t_{sfx}")
    nc.scalar.activation(t_f[:], xt[:], AF.Tanh)
    nc.vector.tensor_copy(gt(1)[:], t_f[:])
    g2_f = fp.tile([P, MACRO], F32, tag="g2", name=f"g2f_{sfx}")
    nc.scalar.activation(g2_f[:], t_f[:], AF.Square)
    nc.vector.tensor_copy(gt(2)[:], g2_f[:])
    g3_f = fp.tile([P, MACRO], F32, tag="g3", name=f"g3f_{sfx}")
    nc.vector.scalar_tensor_tensor(
        g3_f[:], g2_f[:], 0.75, t_f[:], OP.subtract, OP.mult
    )
    nc.scalar.activation(gt(3)[:], g3_f[:], AF.Copy)
    g4_f = fp.tile([P, MACRO], F32, tag="g4", name=f"g4f_{sfx}")
    nc.scalar.activation(g4_f[:], g2_f[:], AF.Square, bias=neg1[:], scale=2.0)
    nc.vector.tensor_copy(gt(4)[:], g4_f[:])
    nc.vector.scalar_tensor_tensor(
        gt(5)[:], g4_f[:], 0.5, t_f[:], OP.subtract, OP.mult
    )
    g6_f = fp.tile([P, MACRO], F32, tag="g6", name=f"g6f_{sfx}")
    nc.scalar.activation(g6_f[:], g3_f[:], AF.Square, scale=4.0)
    nc.scalar.activation(gt(6)[:], g6_f[:], AF.Copy)
    nc.vector.scalar_tensor_tensor(
        gt(7)[:], g6_f[:], 0.5, t_f[:], OP.subtract, OP.mult
    )
    nc.scalar.activation(gt(8)[:], g4_f[:], AF.Square, bias=neg1[:], scale=2.0)


def build_nc(reps=1):
    nc = bacc.Bacc("TRN2", target_bir_lowering=False, debug=False, num_devices=CORES)
    xt_d = nc.dram_tensor("xt", [I, BC], F32, kind="ExternalInput")
    c2_d = nc.dram_tensor("c2", [LEV * I, O], BF16, kind="ExternalInput")
    bias_d = nc.dram_tensor("bias", [P, O], F32, kind="ExternalInput")
    y_d = nc.dram_tensor("y", [BC, O], F32, kind="ExternalOutput")

    NMACROS = NM * reps

    with tile.TileContext(nc) as tc:
        with (
            tc.tile_pool(name="xp", bufs=2) as xp,       # x staging
            tc.tile_pool(name="cp", bufs=1) as cp,       # constants
            tc.tile_pool(name="fp", bufs=2) as fp,       # f32 chain tiles
            tc.tile_pool(name="gp", bufs=1) as gp,       # bf16 G tiles (64 tags)
            tc.tile_pool(name="c2p", bufs=4) as c2p,     # coeff stream
            tc.tile_pool(name="op", bufs=8) as op_,      # psum eviction staging
            tc.tile_pool(name="pp", bufs=1, space="PSUM") as pp,
        ):
            neg1 = cp.tile([P, 1], F32, tag="neg1")
            nc.vector.memset(neg1[:], -1.0)
            bias_t = cp.tile([P, O], F32, tag="bias")
            nc.sync.dma_start(bias_t[:], bias_d[:, :])

            g = {}
            for ib in range(IB):
                _basis(nc, xp, fp, gp, xt_d, neg1, 0, ib, g)

            for gm in range(NMACROS):
                m = gm % NM
                psum = [
                    pp.tile([P, OH], F32, tag=f"ps{j}", name=f"ps{j}_{gm}")
                    for j in range(2 * BT)
                ]
                for k in range(NK):
                    ib, le = divmod(k, LEV)
                    c2t = c2p.tile([P, O], BF16, tag="c2", name=f"c2_{gm}_{k}")
                    nc.sync.dma_start(c2t[:], c2_d[k * P:(k + 1) * P, :])
                    gtile = g[(gm, le + 1, ib)]
                    for bt in range(BT):
                        for oh in range(2):
                            nc.tensor.matmul(
                                psum[bt * 2 + oh][:],
                                gtile[:, bt * P:(bt + 1) * P],
                                c2t[:, oh * OH:(oh + 1) * OH],
                                start=(k == 0),
                                stop=(k == NK - 1),
                            )
                    if le == LEV - 1:
                        # macro gm is done with i-block ib; build its
                        # replacement for macro gm+1 (G tags single-buffered,
                        # WAR on this macro's retired matmuls)
                        for lev in range(1, LEV + 1):
                            g.pop((gm - 1, lev, ib), None)
                        if gm + 1 < NMACROS:
                            _basis(nc, xp, fp, gp, xt_d, neg1, gm + 1, ib, g)

                for bt in range(BT):
                    for oh in range(2):
                        ob = op_.tile([P, OH], F32, tag="ob", name=f"ob_{gm}_{bt}_{oh}")
                        nc.vector.tensor_tensor(
                            ob[:],
                            psum[bt * 2 + oh][:],
                            bias_t[:, oh * OH:(oh + 1) * OH],
                            OP.add,
                        )
                        nc.scalar.dma_start(
                            y_d[
                                m * MACRO + bt * P:m * MACRO + (bt + 1) * P,
                                oh * OH:(oh + 1) * OH,
                            ],
                            ob[:],
                        )
    nc.compile()
    return nc


def _prep_coeffs(cheby_coeffs):
    cp = np.einsum("ed,iod->ioe", _A, cheby_coeffs.astype(np.float64))  # (I, O, 9)
    bias = cp[:, :, 0].sum(axis=0)                                     # (O,)
    arr = cp[:, :, 1:].reshape(IB, P, O, LEV).transpose(0, 3, 1, 2)    # (ib, e, r, o)
    c2 = np.ascontiguousarray(arr.reshape(LEV * I, O)).astype(mybir.dt.np(BF16))
    bias_rep = np.ascontiguousarray(
        np.broadcast_to(bias[None, :], (P, O))
    ).astype(np.float32)
    return c2, bias_rep


def kernel(x: np.ndarray, cheby_coeffs: np.ndarray) -> np.ndarray:
    assert x.shape == (B, I) and cheby_coeffs.shape == (I, O, D1)
    if "nc" not in _CACHE:
        _CACHE["nc"] = build_nc()
    nc = _CACHE["nc"]

    xt = np.ascontiguousarray(x.T.astype(np.float32, copy=False))      # (I, B)
    c2, bias_rep = _prep_coeffs(cheby_coeffs)
    in_maps = [
        {
            "xt": np.ascontiguousarray(xt[:, c * BC:(c + 1) * BC]),
            "c2": c2,
            "bias": bias_rep,
        }
        for c in range(CORES)
    ]
    global _last_in_maps
    _last_in_maps = in_maps
    res = run_bass_kernel_spmd(nc, in_maps, core_ids=list(range(CORES)))
    return np.concatenate([res.results[c]["y"] for c in range(CORES)], axis=0)


# revision 5
# speedup vs baseline: 1.0253x; 1.0253x over previous
"""ChebyKAN layer on 8 Trainium2 NeuronCores.

y[b,o] = sum_{i,d} T_d(tanh(x[b,i])) * coeffs[i,o,d]

The Chebyshev basis is re-parameterized (exact 9x9 linear transform of the
coefficients on host) into products of Chebyshev values built with ACT
squares + DVE fused ops per element:
  G0=1, G1=t, G2=t^2, G3q=(G2-3/4)t=T3/4, G4=(2G2-1)^2=T2^2,
  G5q=(G4-1/2)t=(T5+T3)/4, G6=(4G3q)^2=T3^2, G7q=(G6-1/2)t=(T7+T5)/4,
  G8=(2G4-1)^2=T4^2

G0=1 contributes a batch-independent bias sum_i c'[i,o,0], folded in on the
host and added during PSUM eviction — the device contraction is only levels
1..8: a (B x 8192) @ (8192 x 1024) matmul in bf16 (1 cycle/row, fast
weight load), accumulated in fp32 PSUM over 64 contraction blocks.

Layout per core (2048 batch rows): 4 macros of 512 rows. Per macro all 64
G-tiles [128, 512] bf16 are resident; the k-loop is i-block-major and for
each k streams one [128, 1024] bf16 coeff block, issuing 4(bt) x 2(oh)
matmuls into 8 PSUM banks. Each stationary G block feeds 2 consecutive
matmuls. Basis for macro m+1 is emitted i-block-by-i-block as macro m's
k-loop retires the matching i-block, so ACT/DVE overlap the PE stream.

Sharding: data-parallel over batch (2048 rows/core), coeffs replicated.
"""

import numpy as np
import concourse.mybir as mybir
import concourse.tile as tile
from concourse import bacc
from concourse.bass_utils import run_bass_kernel_spmd

B, I, O = 16384, 1024, 1024
D1 = 9
CORES = 8
BC = B // CORES            # 2048 batch rows per core
P = 128
MACRO = 512                # batch rows per macro generation
NM = BC // MACRO           # 4 macros
OH = 512                   # psum bank width
IB = I // P                # 8 i-blocks
LEV = 8                    # device levels (G1..G8)
NK = LEV * IB              # 64 contraction blocks
BT = MACRO // P            # 4 batch subtiles per macro

F32 = mybir.dt.float32
BF16 = mybir.dt.bfloat16
AF = mybir.ActivationFunctionType
OP = mybir.AluOpType

_CACHE = {}
_last_in_maps = None

# G_k = sum_d M[k,d] T_d  (exact); host solves M^T C' = C
_M = np.zeros((9, 9))
_M[0, 0] = 1; _M[1, 1] = 1
_M[2, 0] = .5; _M[2, 2] = .5
_M[3, 3] = .25
_M[4, 0] = .5; _M[4, 4] = .5
_M[5, 3] = .25; _M[5, 5] = .25
_M[6, 0] = .5; _M[6, 6] = .5
_M[7, 5] = .25; _M[7, 7] = .25
_M[8, 0] = .5; _M[8, 8] = .5
_A = np.linalg.inv(_M.T)


def _basis(nc, xp, fp, gp, xt_d, neg1, gm, ib, g):
    """Emit G1..G8 [128, MACRO] bf16 tiles for (macro gm, i-block ib)."""
    m = gm % NM
    sfx = f"{gm}_{ib}"
    xt = xp.tile([P, MACRO], F32, tag="xt", name=f"xt_{sfx}")
    nc.sync.dma_start(xt[:], xt_d[ib * P:(ib + 1) * P, m * MACRO:(m + 1) * MACRO])

    def gt(lev):
        w = gp.tile([P, MACRO], BF16, tag=f"g{lev}_{ib}", name=f"g{lev}_{ib}_{gm}")
        g[(gm, lev, ib)] = w
        return w

    t_f = fp.tile([P, MACRO], F32, tag="t", name=f"t_{sfx}")
    nc.scalar.activation(t_f[:], xt[:], AF.Tanh)
    nc.vector.tensor_copy(gt(1)[:], t_f[:])
    g2_f = fp.tile([P, MACRO], F32, tag="g2", name=f"g2f_{sfx}")
    nc.scalar.activation(g2_f[:], t_f[:], AF.Square)
    nc.vector.tensor_copy(gt(2)[:], g2_f[:])
    g3_f = fp.tile([P, MACRO], F32, tag="g3", name=f"g3f_{sfx}")
    nc.vector.scalar_tensor_tensor(
        g3_f[:], g2_f[:], 0.75, t_f[:], OP.subtract, OP.mult
    )
    nc.scalar.activation(gt(3)[:], g3_f[:], AF.Copy)
    g4_f = fp.tile([P, MACRO], F32, tag="g4", name=f"g4f_{sfx}")
    nc.scalar.activation(g4_f[:], g2_f[:], AF.Square, bias=neg1[:], scale=2.0)
    nc.vector.tensor_copy(gt(4)[:], g4_f[:])
    nc.vector.scalar_tensor_tensor(
        gt(5)[:], g4_f[:], 0.5, t_f[:], OP.subtract, OP.mult
    )
    g6_f = fp.tile([P, MACRO], F32, tag="g6", name=f"g6f_{sfx}")
    nc.scalar.activation(g6_f[:], g3_f[:], AF.Square, scale=4.0)
    nc.scalar.activation(gt(6)[:], g6_f[:], AF.Copy)
    nc.vector.scalar_tensor_tensor(
        gt(7)[:], g6_f[:], 0.5, t_f[:], OP.subtract, OP.mult
    )
    nc.scalar.activation(gt(8)[:], g4_f[:], AF.Square, bias=neg1[:], scale=2.0)


def build_nc(reps=1):
    nc = bacc.Bacc("TRN2", target_bir_lowering=False, debug=False, num_devices=CORES)
    xt_d = nc.dram_tensor("xt", [I, BC], F32, kind="ExternalInput")
    c2_d = nc.dram_tensor("c2", [LEV * I, O], BF16, kind="ExternalInput")
    bias_d = nc.dram_tensor("bias", [P, O], F32, kind="ExternalInput")
    y_d = nc.dram_tensor("y", [BC, O], F32, kind="ExternalOutput")

    NMACROS = NM * reps

    with tile.TileContext(nc) as tc:
        with (
            tc.tile_pool(name="xp", bufs=2) as xp,       # x staging
            tc.tile_pool(name="cp", bufs=1) as cp,       # constants
            tc.tile_pool(name="fp", bufs=2) as fp,       # f32 chain tiles
            tc.tile_pool(name="gp", bufs=1) as gp,       # bf16 G tiles (64 tags)
            tc.tile_pool(name="c2p", bufs=6) as c2p,     # coeff stream
            tc.tile_pool(name="op", bufs=8) as op_,      # psum eviction staging
            tc.tile_pool(name="pp", bufs=1, space="PSUM") as pp,
        ):
            neg1 = cp.tile([P, 1], F32, tag="neg1")
            nc.vector.memset(neg1[:], -1.0)

            PF = 6  # c2 prefetch depth (== c2p bufs)
            c2_tiles = {}

            def pf_c2(gidx):
                gm2, k2 = divmod(gidx, NK)
                t = c2p.tile([P, O], BF16, tag="c2", name=f"c2_{gm2}_{k2}")
                nc.sync.dma_start(t[:], c2_d[k2 * P:(k2 + 1) * P, :])
                c2_tiles[gidx] = t

            g = {}
            # first i-block's basis first so its x DMA leads the sync ring,
            # then the first coeff tiles, then the rest of macro-0's basis
            _basis(nc, xp, fp, gp, xt_d, neg1, 0, 0, g)
            for j in range(PF):
                pf_c2(j)
            for ib in range(1, IB):
                _basis(nc, xp, fp, gp, xt_d, neg1, 0, ib, g)
            bias_t = cp.tile([P, O], F32, tag="bias")
            nc.sync.dma_start(bias_t[:], bias_d[:, :])

            for gm in range(NMACROS):
                m = gm % NM
                psum = [
                    pp.tile([P, OH], F32, tag=f"ps{j}", name=f"ps{j}_{gm}")
                    for j in range(2 * BT)
                ]
                for k in range(NK):
                    ib, le = divmod(k, LEV)
                    c2t = c2_tiles.pop(gm * NK + k)
                    gtile = g[(gm, le + 1, ib)]
                    for bt in range(BT):
                        for oh in range(2):
                            nc.tensor.matmul(
                                psum[bt * 2 + oh][:],
                                gtile[:, bt * P:(bt + 1) * P],
                                c2t[:, oh * OH:(oh + 1) * OH],
                                start=(k == 0),
                                stop=(k == NK - 1),
                            )
                    if gm * NK + k + PF < NMACROS * NK:
                        pf_c2(gm * NK + k + PF)
                    if le == LEV - 1:
                        # macro gm is done with i-block ib; build its
                        # replacement for macro gm+1 (G tags single-buffered,
                        # WAR on this macro's retired matmuls)
                        for lev in range(1, LEV + 1):
                            g.pop((gm - 1, lev, ib), None)
                        if gm + 1 < NMACROS:
                            _basis(nc, xp, fp, gp, xt_d, neg1, gm + 1, ib, g)

                for bt in range(BT):
                    for oh in range(2):
                        ob = op_.tile([P, OH], F32, tag="ob", name=f"ob_{gm}_{bt}_{oh}")
                        nc.vector.tensor_tensor(
                            ob[:],
                            psum[bt * 2 + oh][:],
                            bias_t[:, oh * OH:(oh + 1) * OH],
                            OP.add,
                        )
                        nc.scalar.dma_start(
                            y_d[
                                m * MACRO + bt * P:m * MACRO + (bt + 1) * P,
                                oh * OH:(oh + 1) * OH,
                            ],
                            ob[:],
                        )
    nc.compile()
    return nc


def _prep_coeffs(cheby_coeffs):
    cp = np.einsum("ed,iod->ioe", _A, cheby_coeffs.astype(np.float64))  # (I, O, 9)
    bias = cp[:, :, 0].sum(axis=0)                                     # (O,)
    arr = cp[:, :, 1:].reshape(IB, P, O, LEV).transpose(0, 3, 1, 2)    # (ib, e, r, o)
    c2 = np.ascontiguousarray(arr.reshape(LEV * I, O)).astype(mybir.dt.np(BF16))
    bias_rep = np.ascontiguousarray(
        np.broadcast_to(bias[None, :], (P, O))
    ).astype(np.float32)
    return c2, bias_rep


def kernel(x: np.ndarray, cheby_coeffs: np.ndarray) -> np.ndarray:
    assert x.shape == (B, I) and cheby_coeffs.shape == (I, O, D1)
    if "nc" not in _CACHE:
        _CACHE["nc"] = build_nc()
    nc = _CACHE["nc"]

    xt = np.ascontiguousarray(x.T.astype(np.float32, copy=False))      # (I, B)
    c2, bias_rep = _prep_coeffs(cheby_coeffs)
    in_maps = [
        {
            "xt": np.ascontiguousarray(xt[:, c * BC:(c + 1) * BC]),
            "c2": c2,
            "bias": bias_rep,
        }
        for c in range(CORES)
    ]
    global _last_in_maps
    _last_in_maps = in_maps
    res = run_bass_kernel_spmd(nc, in_maps, core_ids=list(range(CORES)))
    return np.concatenate([res.results[c]["y"] for c in range(CORES)], axis=0)
